# revision 11
# baseline (speedup 1.0000x reference)
"""Causal self-attention, tensor-parallel over heads across 8 TRN2 NeuronCores.

B=2, T=2048, C=1024, H=16 heads, D=64. Each core owns 2 heads (128 cols of C)
for both batches. Layout is head-paired: qT/kT tiles hold h0 dims on
partitions 0-63 and h1 dims on 64-127, so the two heads' score matmuls run
concurrently on disjoint PE row groups (no operand duplication). Per-batch
AllToAlls redistribute the column-sharded attention output y^T (plus softmax
normalizer rows) into token shards; batch-0's A2A overlaps batch-1 compute and
batch-0's Wo contraction overlaps the batch-1 A2A. bf16 matmuls, fp32 PSUM.
"""

import sys

sys.path.insert(0, "/opt/trn_rl_repo")

import numpy as np
import ml_dtypes

import concourse.bass as bass
import concourse.bacc as bacc
import concourse.mybir as mybir
from concourse.tile import TileContext
from concourse import bass_utils

BF16 = mybir.dt.bfloat16
F32 = mybir.dt.float32
NPBF16 = ml_dtypes.bfloat16

B, T, C, H, D = 2, 2048, 1024, 16, 64
NCORES = 8
HL = H // NCORES          # heads per core = 2
COLS = HL * D             # 128 head-cols per core
KT = C // 128             # 8 contraction k-tiles
NCH = T // 512            # 4 query chunks of 512 per batch
NT = T // 128             # 16 key tiles of 128 per batch
VW = D + 1                # 65: v columns + ones column
SH = 136                  # A2A shard rows: h0 (64y + r), h1 (64y + r), pad 4KB
ROWS_PER_CORE = B * T // NCORES  # 512 output rows per core

SCALE = 1.0 / np.sqrt(np.float32(D))

Exp = mybir.ActivationFunctionType.Exp


def build_nc():
    nc = bacc.Bacc(
        "TRN2",
        target_bir_lowering=False,
        debug=False,
        enable_asserts=False,
        num_devices=NCORES,
    )
    xT = nc.dram_tensor("xT", [C, B * T], BF16, kind="ExternalInput")
    # weights pre-tiled on host to [128, k-tile blocks] for contiguous DMA
    wq = nc.dram_tensor("wq", [128, KT * COLS], BF16, kind="ExternalInput")
    wk = nc.dram_tensor("wk", [128, KT * COLS], BF16, kind="ExternalInput")
    wv = nc.dram_tensor("wv", [128, KT * COLS], BF16, kind="ExternalInput")
    wo = nc.dram_tensor("wo", [128, KT * C], BF16, kind="ExternalInput")
    bqk = nc.dram_tensor("bqk", [COLS, 2], F32, kind="ExternalInput")
    bv4 = nc.dram_tensor("bv4", [1, 512], BF16, kind="ExternalInput")
    bo = nc.dram_tensor("bo", [1, C], BF16, kind="ExternalInput")
    # 0/1 upper triangle (incl diag): tri[k, q] = 1 iff q >= k
    tri = nc.dram_tensor("tri", [128, 128], BF16, kind="ExternalInput")
    emat = nc.dram_tensor("emat", [16, KT * 128], BF16, kind="ExternalInput")
    # one A2A per batch; shard for dest core d = rows [d*SH, d*SH+130):
    # h0 y (64) + h0 r, h1 y (64) + h1 r; 256 cols = dest's tokens of batch b
    send = [nc.dram_tensor(f"a2a_send{b}", [NCORES * SH, 256], BF16) for b in range(B)]
    recv = [nc.dram_tensor(f"a2a_recv{b}", [NCORES * SH, 256], BF16) for b in range(B)]
    wsend = nc.dram_tensor("warm_send", [NCORES * 2, 512], BF16)
    wrecv = nc.dram_tensor("warm_recv", [NCORES * 2, 512], BF16)
    out = nc.dram_tensor("out", [ROWS_PER_CORE, C], F32, kind="ExternalOutput")

    add = mybir.AluOpType.add
    mult = mybir.AluOpType.mult

    def proj_qk(b, n, x_sb, psA, cs):
        """q and k projections for query chunk n of batch b (both heads)."""
        col = b * T + n * 512
        for wsb, dsb, bcol in ((cs["wq"], cs["qTd"], 0), (cs["wk"], cs["kTd"], 1)):
            ps = psA.tile([128, 512], F32, tag="psA", name="psA", bufs=2)
            for k in range(KT):
                nc.tensor.matmul(
                    ps[:],
                    wsb[:, k * COLS : (k + 1) * COLS],
                    x_sb[k][:, col : col + 512],
                    start=(k == 0),
                    stop=(k == KT - 1),
                )
            nc.vector.tensor_scalar(
                dsb[:, col : col + 512],
                ps[:],
                cs["bqk"][:, bcol : bcol + 1],
                None,
                add,
            )

    def proj_v(b, g, x_sb, psA, cs):
        """v projection for token-tile group g (4 tiles of 128) of batch b."""
        ps = psA.tile([128, 512], F32, tag="psA", name="psAv", bufs=2)
        for ml in range(4):
            m = g * 4 + ml
            col = b * T + m * 128
            for k in range(KT):
                nc.tensor.matmul(
                    ps[:, ml * 128 : (ml + 1) * 128],
                    x_sb[k][:, col : col + 128],
                    cs["wv"][:, k * COLS : (k + 1) * COLS],
                    # start=True clears has_written for the WHOLE bank, so only
                    # the very first matmul of the packed group may set it
                    start=(ml == 0 and k == 0),
                    stop=False,
                )
        nc.tensor.matmul(
            ps[:], cs["ones"][0:1, :], cs["bv4"][0:1, :], start=False, stop=True
        )
        vi = (b * NT + g * 4) * HL
        nc.vector.tensor_copy(
            out=cs["v"][:, vi : vi + 4 * HL, 0:D],
            in_=ps[:].rearrange("p (m h d) -> p (m h) d", h=HL, d=D),
        )

    def attention(b, n, ptp, psS, psY, nrm, cs, send_instrs):
        """Causal attention for query chunk n of batch b, both heads."""
        qcol = b * T + n * 512
        qTd, kTd = cs["qTd"], cs["kTd"]
        last = 4 * n + 3
        py = [
            psY.tile([VW, 512], F32, tag=f"psY{h}", name=f"psY{h}", bufs=1)
            for h in range(HL)
        ]
        pts = []

        def emit_av(m):
            pt, lo = pts[m]
            for h in range(HL):
                vi = (b * NT + m) * HL + h
                nc.tensor.matmul(
                    py[h][:, lo:512],
                    cs["v"][:, vi : vi + 1, :],
                    pt[:, h * 512 + lo : (h + 1) * 512],
                    start=(m == 0),
                    stop=(m == last),
                )

        for m in range(last + 1):
            j = m - 4 * n  # >= 0 on the block-diagonal
            lo = max(j, 0) * 128
            ps2 = psS.tile([128, 1024], F32, tag="psS2", name="psS2", bufs=2)
            pt = ptp.tile([128, 1024], BF16, tag="pt", name="pt", bufs=5)
            kcol = b * T + m * 128
            for h in range(HL):
                nc.tensor.matmul(
                    ps2[:, h * 512 + lo : (h + 1) * 512],
                    kTd[h * D : (h + 1) * D, kcol : kcol + 128],
                    qTd[h * D : (h + 1) * D, qcol + lo : qcol + 512],
                    start=True,
                    stop=True,
                )
            if lo == 0:
                nc.scalar.activation(pt[:], ps2[:], Exp, scale=float(SCALE))
            else:
                for h in range(HL):
                    nc.scalar.activation(
                        pt[:, h * 512 + lo : (h + 1) * 512],
                        ps2[:, h * 512 + lo : (h + 1) * 512],
                        Exp,
                        scale=float(SCALE),
                    )
            if j >= 0:
                # zero the strict upper triangle of the diagonal 128-block
                for h in range(HL):
                    blk = pt[:, h * 512 + lo : h * 512 + lo + 128]
                    nc.vector.tensor_tensor(blk, blk, cs["tri"][:], mult)
            pts.append((pt, lo))
            if m > 0:
                emit_av(m - 1)
        emit_av(last)

        # ship unnormalized y^T + r rows; receiver divides
        yn = [
            nrm.tile([VW, 512], BF16, tag=f"yn{h}", name=f"yn{h}", bufs=2)
            for h in range(HL)
        ]
        for h in range(HL):
            nc.vector.tensor_copy(out=yn[h][:], in_=py[h][:])
        for p in range(2):
            dst = 2 * n + p
            for h in range(HL):
                si = nc.sync.dma_start(
                    out=send[b][dst * SH + h * VW : dst * SH + (h + 1) * VW, :],
                    in_=yn[h][:, p * 256 : (p + 1) * 256],
                )
                send_instrs.append(si)

    def a2a(b):
        nc.gpsimd.collective_compute(
            "AllToAll",
            mybir.AluOpType.bypass,
            replica_groups=[list(range(NCORES))],
            ins=[send[b][:]],
            outs=[recv[b][:]],
        )

    def stage_c(b, cp, psC, cs):
        """Wo row-shard contraction for this core's 256 tokens of batch b."""
        rv = recv[b].rearrange("(s r) c -> r s c", s=NCORES)
        y_sb = cp.tile([128, KT * 256], BF16, tag="ysb", name=f"ysb{b}", bufs=2)
        # y columns: chunk k partitions 0-63 = sender k h0 dims, 64-127 = h1;
        # two queues so the halves unpack in parallel
        nc.scalar.dma_start(
            out=y_sb[0:64, :].rearrange("d (s c) -> d s c", s=NCORES),
            in_=rv[0:64, :, :],
        )
        nc.sync.dma_start(
            out=y_sb[64:128, :].rearrange("d (s c) -> d s c", s=NCORES),
            in_=rv[VW + 0 : VW + 64, :, :],
        )
        # SBUF-destination DMAs need the partition dim outermost in the AP
        rv_s = recv[b].rearrange("(s r) c -> s r c", s=NCORES)
        rsb = cp.tile([16, 256], BF16, tag="rsb", name=f"rsb{b}", bufs=2)
        nc.scalar.dma_start(
            out=rsb[0:8, :].rearrange("s (o c) -> s o c", o=1),
            in_=rv_s[:, 64:65, :],
        )
        nc.scalar.dma_start(
            out=rsb[8:16, :].rearrange("s (o c) -> s o c", o=1),
            in_=rv_s[:, VW + 64 : VW + 65, :],
        )
        rf = cp.tile([16, 256], F32, tag="rf", name=f"rf{b}", bufs=2)
        nc.vector.tensor_copy(out=rf[:], in_=rsb[:])
        invf = cp.tile([16, 256], F32, tag="invf", name=f"invf{b}", bufs=2)
        nc.vector.reciprocal_approx_fast(out=invf[:], in_=rf[:])
        inv = cp.tile([16, 256], BF16, tag="inv", name=f"inv{b}", bufs=2)
        nc.vector.tensor_copy(out=inv[:], in_=invf[:])

        yn_sb = cp.tile([128, KT * 256], BF16, tag="ynsb", name=f"ynsb{b}", bufs=2)
        for k in range(KT):
            sc = psC.tile([128, 256], F32, tag="psN", name="psN", bufs=2)
            nc.tensor.matmul(
                sc[:], cs["emat"][:, k * 128 : (k + 1) * 128], inv[:],
                start=True, stop=True,
            )
            nc.vector.tensor_tensor(
                yn_sb[:, k * 256 : (k + 1) * 256],
                y_sb[:, k * 256 : (k + 1) * 256],
                sc[:],
                mult,
            )
        for r in range(2):
            for o in range(2):
                pc = psC.tile([128, 512], F32, tag="psC", name="psC", bufs=2)
                for k in range(KT):
                    nc.tensor.matmul(
                        pc[:],
                        yn_sb[:, k * 256 + r * 128 : k * 256 + (r + 1) * 128],
                        cs["wo"][:, k * C + o * 512 : k * C + (o + 1) * 512],
                        start=(k == 0),
                        stop=False,
                    )
                nc.tensor.matmul(
                    pc[:], cs["ones"][0:1, :], cs["bo"][0:1, o * 512 : (o + 1) * 512],
                    start=False, stop=True,
                )
                osb = cp.tile([128, 512], F32, tag="osb", name="osb", bufs=2)
                nc.vector.tensor_copy(out=osb[:], in_=pc[:])
                nc.sync.dma_start(
                    out=out[b * 256 + r * 128 : b * 256 + (r + 1) * 128,
                            o * 512 : (o + 1) * 512],
                    in_=osb[:],
                )

    with TileContext(nc) as tc:
        with tc.tile_pool(name="persist", bufs=1) as pp:
            cs = {}
            # warmup collective first on the gpsimd queue: starts the cross-core
            # barrier + CC init while projections run
            nc.gpsimd.collective_compute(
                "AllToAll",
                mybir.AluOpType.bypass,
                replica_groups=[list(range(NCORES))],
                ins=[wsend[:]],
                outs=[wrecv[:]],
            )
            # tiny constants first: a late-landing constant can head-block the
            # PE queue
            cs["ones"] = pp.tile([1, 128], BF16, tag="ones", name="ones")
            nc.vector.memset(cs["ones"][:], 1.0)
            cs["bqk"] = pp.tile([COLS, 2], F32, tag="bqk", name="bqk")
            nc.sync.dma_start(out=cs["bqk"][:], in_=bqk[:])
            cs["bv4"] = pp.tile([1, 512], BF16, tag="bv4", name="bv4")
            nc.sync.dma_start(out=cs["bv4"][:], in_=bv4[:])
            cs["bo"] = pp.tile([1, C], BF16, tag="bo", name="bo")
            nc.scalar.dma_start(out=cs["bo"][:], in_=bo[:])
            cs["tri"] = pp.tile([128, 128], BF16, tag="tri", name="tri")
            nc.scalar.dma_start(out=cs["tri"][:], in_=tri[:])
            cs["emat"] = pp.tile([16, KT * 128], BF16, tag="emat", name="emat")
            nc.gpsimd.dma_start(out=cs["emat"][:], in_=emat[:])

            # weights: one per queue so each lands fast, before x
            cs["wq"] = pp.tile([128, KT * COLS], BF16, tag="wq", name="wq")
            cs["wk"] = pp.tile([128, KT * COLS], BF16, tag="wk", name="wk")
            cs["wv"] = pp.tile([128, KT * COLS], BF16, tag="wv", name="wv")
            nc.sync.dma_start(out=cs["wq"][:], in_=wq[:])
            nc.scalar.dma_start(out=cs["wk"][:], in_=wk[:])
            nc.gpsimd.dma_start(out=cs["wv"][:], in_=wv[:])

            with tc.tile_pool(name="xp", bufs=1) as xp:
                # x split per (token-chunk, k-tile) across 3 DMA trigger queues,
                # first-consumed chunks first
                x_sb = [
                    xp.tile([128, B * T], BF16, tag=f"x{k}", name=f"x{k}")
                    for k in range(KT)
                ]
                xq = [nc.sync, nc.scalar, nc.gpsimd]
                for c in range(4):
                    for k in range(KT):
                        xq[(c * KT + k) % 3].dma_start(
                            out=x_sb[k][:, c * 1024 : (c + 1) * 1024],
                            in_=xT[k * 128 : (k + 1) * 128, c * 1024 : (c + 1) * 1024],
                        )

                cs["qTd"] = pp.tile([128, B * T], BF16, tag="qTd", name="qTd")
                cs["kTd"] = pp.tile([128, B * T], BF16, tag="kTd", name="kTd")
                cs["v"] = pp.tile([128, B * NT * HL, VW], BF16, tag="v", name="v")
                nc.gpsimd.memset(cs["v"][:], 1.0)  # presets the ones columns

                # wo loaded last (not needed until stage C)
                cs["wo"] = pp.tile([128, KT * C], BF16, tag="wo", name="wo")
                nc.scalar.dma_start(out=cs["wo"][:], in_=wo[:])

                send_instrs = []
                with tc.tile_pool(name="psA", bufs=2, space="PSUM") as psA, \
                     tc.tile_pool(name="pt", bufs=5) as ptp, \
                     tc.tile_pool(name="psS", bufs=1, space="PSUM") as psS, \
                     tc.tile_pool(name="psY", bufs=1, space="PSUM") as psY, \
                     tc.tile_pool(name="nrm", bufs=2) as nrm:
                    for b in range(B):
                        # chunk-major: everything needing x-chunk c lands
                        # together; attention interleaves to cover DMA waits
                        proj_qk(b, 0, x_sb, psA, cs)
                        proj_qk(b, 1, x_sb, psA, cs)
                        proj_v(b, 0, x_sb, psA, cs)
                        proj_v(b, 1, x_sb, psA, cs)
                        attention(b, 0, ptp, psS, psY, nrm, cs, send_instrs)
                        attention(b, 1, ptp, psS, psY, nrm, cs, send_instrs)
                        proj_qk(b, 2, x_sb, psA, cs)
                        proj_qk(b, 3, x_sb, psA, cs)
                        proj_v(b, 2, x_sb, psA, cs)
                        proj_v(b, 3, x_sb, psA, cs)
                        attention(b, 2, ptp, psS, psY, nrm, cs, send_instrs)
                        attention(b, 3, ptp, psS, psY, nrm, cs, send_instrs)
                        a2a(b)

            with tc.tile_pool(name="cp", bufs=1) as cp, \
                 tc.tile_pool(name="psC", bufs=2, space="PSUM") as psC:
                stage_c(0, cp, psC, cs)
                stage_c(1, cp, psC, cs)
    nc.compile()
    return nc


def make_in_maps(x, mask, Wq, bq, Wk, bk, Wv, bv, Wo, bo):
    xT = np.ascontiguousarray(
        x.astype(np.float32).transpose(2, 0, 1).reshape(C, B * T)
    ).astype(NPBF16)
    tri_np = (
        np.arange(128)[None, :] >= np.arange(128)[:, None]
    ).astype(NPBF16)

    def pretile(w):
        # [C, width] -> [128, KT*width] with k-tile blocks along free axis
        width = w.shape[1]
        return np.ascontiguousarray(
            w.reshape(KT, 128, width).transpose(1, 0, 2).reshape(128, KT * width)
        )

    wo_b = pretile(Wo.astype(NPBF16))
    # emat[h*8+s, k*128+p] = 1 iff chunk k's partition p takes sender s, head h
    rows = np.arange(16)
    cols = np.arange(KT * 128)
    emat_np = (
        ((rows % 8)[:, None] == cols[None, :] // 128)
        & ((rows // 8)[:, None] == (cols[None, :] % 128) // 64)
    ).astype(NPBF16)
    bo_b = bo.reshape(1, C).astype(NPBF16)
    in_maps = []
    for c in range(NCORES):
        cslice = slice(c * COLS, (c + 1) * COLS)
        in_maps.append(
            {
                "xT": xT,
                "wq": pretile(Wq[:, cslice].astype(NPBF16)),
                "wk": pretile(Wk[:, cslice].astype(NPBF16)),
                "wv": pretile(Wv[:, cslice].astype(NPBF16)),
                "wo": wo_b,
                "bqk": np.stack([bq[cslice], bk[cslice]], axis=1).astype(
                    np.float32
                ),
                "bv4": np.tile(bv[cslice].astype(NPBF16), 4).reshape(1, 512),
                "bo": bo_b,
                "tri": tri_np,
                "emat": emat_np,
            }
        )
    return in_maps


_CACHED_NC = None


def run(inputs, trace=False, **kw):
    global _CACHED_NC
    if _CACHED_NC is None:
        _CACHED_NC = build_nc()
    in_maps = make_in_maps(**inputs)
    res = bass_utils.run_bass_kernel_spmd(
        _CACHED_NC, in_maps, core_ids=list(range(NCORES)), trace=trace, **kw
    )
    outs = [np.asarray(res.results[c]["out"]) for c in range(NCORES)]
    full = np.empty((B, T, C), np.float32)
    for j in range(NCORES):
        full[0, 256 * j : 256 * (j + 1)] = outs[j][0:256]
        full[1, 256 * j : 256 * (j + 1)] = outs[j][256:512]
    return full, res


def kernel(**inputs):
    full, _ = run(inputs, trace=False)
    return full


# revision 12
# speedup vs baseline: 1.0976x; 1.0976x over previous
"""Causal self-attention, tensor-parallel over heads across 8 TRN2 NeuronCores.

B=2, T=2048, C=1024, H=16 heads, D=64. Each core owns 2 heads (128 cols of C)
for both batches. Layout is head-paired: qT/kT tiles hold h0 dims on
partitions 0-63 and h1 dims on 64-127, so the two heads' score matmuls run
concurrently on disjoint PE row groups (no operand duplication). Per-batch
AllToAlls redistribute the column-sharded attention output y^T (plus softmax
normalizer rows) into token shards; batch-0's A2A overlaps batch-1 compute and
batch-0's Wo contraction overlaps the batch-1 A2A. bf16 matmuls, fp32 PSUM.
"""

import sys

sys.path.insert(0, "/opt/trn_rl_repo")

import numpy as np
import ml_dtypes

import concourse.bass as bass
import concourse.bacc as bacc
import concourse.mybir as mybir
from concourse.tile import TileContext
from concourse import bass_utils

BF16 = mybir.dt.bfloat16
F32 = mybir.dt.float32
NPBF16 = ml_dtypes.bfloat16

B, T, C, H, D = 2, 2048, 1024, 16, 64
NCORES = 8
HL = H // NCORES          # heads per core = 2
COLS = HL * D             # 128 head-cols per core
KT = C // 128             # 8 contraction k-tiles
NCH = T // 512            # 4 query chunks of 512 per batch
NT = T // 128             # 16 key tiles of 128 per batch
VW = D + 1                # 65: v columns + ones column
SH = 136                  # A2A shard rows: h0 (64y + r), h1 (64y + r), pad 4KB
ROWS_PER_CORE = B * T // NCORES  # 512 output rows per core

SCALE = 1.0 / np.sqrt(np.float32(D))

Exp = mybir.ActivationFunctionType.Exp


def build_nc():
    nc = bacc.Bacc(
        "TRN2",
        target_bir_lowering=False,
        debug=False,
        enable_asserts=False,
        num_devices=NCORES,
    )
    xT = nc.dram_tensor("xT", [C, B * T], BF16, kind="ExternalInput")
    # weights pre-tiled on host to [128, k-tile blocks] for contiguous DMA
    wq = nc.dram_tensor("wq", [128, KT * COLS], BF16, kind="ExternalInput")
    wk = nc.dram_tensor("wk", [128, KT * COLS], BF16, kind="ExternalInput")
    wv = nc.dram_tensor("wv", [128, KT * COLS], BF16, kind="ExternalInput")
    wo = nc.dram_tensor("wo", [128, KT * C], BF16, kind="ExternalInput")
    bqk = nc.dram_tensor("bqk", [COLS, 2], F32, kind="ExternalInput")
    bv4 = nc.dram_tensor("bv4", [1, 512], BF16, kind="ExternalInput")
    bo = nc.dram_tensor("bo", [1, C], BF16, kind="ExternalInput")
    # 0/1 upper triangle (incl diag): tri[k, q] = 1 iff q >= k
    tri = nc.dram_tensor("tri", [128, 128], BF16, kind="ExternalInput")
    emat = nc.dram_tensor("emat", [16, KT * 128], BF16, kind="ExternalInput")
    # one A2A per batch; shard for dest core d = rows [d*SH, d*SH+130):
    # h0 y (64) + h0 r, h1 y (64) + h1 r; 256 cols = dest's tokens of batch b
    send = [nc.dram_tensor(f"a2a_send{b}", [NCORES * SH, 256], BF16) for b in range(B)]
    recv = [nc.dram_tensor(f"a2a_recv{b}", [NCORES * SH, 256], BF16) for b in range(B)]
    wsend = nc.dram_tensor("warm_send", [NCORES * 2, 512], BF16)
    wrecv = nc.dram_tensor("warm_recv", [NCORES * 2, 512], BF16)
    out = nc.dram_tensor("out", [ROWS_PER_CORE, C], F32, kind="ExternalOutput")

    add = mybir.AluOpType.add
    mult = mybir.AluOpType.mult

    def proj_qk(b, n, x_sb, psA, cs):
        """q and k projections for query chunk n of batch b (both heads)."""
        col = b * T + n * 512
        for wsb, dsb, bcol in ((cs["wq"], cs["qTd"], 0), (cs["wk"], cs["kTd"], 1)):
            ps = psA.tile([128, 512], F32, tag="psA", name="psA", bufs=2)
            for k in range(KT):
                nc.tensor.matmul(
                    ps[:],
                    wsb[:, k * COLS : (k + 1) * COLS],
                    x_sb[k][:, col : col + 512],
                    start=(k == 0),
                    stop=(k == KT - 1),
                )
            nc.vector.tensor_scalar(
                dsb[:, col : col + 512],
                ps[:],
                cs["bqk"][:, bcol : bcol + 1],
                None,
                add,
            )

    def proj_v(b, g, x_sb, psA, cs):
        """v projection for token-tile group g (4 tiles of 128) of batch b."""
        ps = psA.tile([128, 512], F32, tag="psA", name="psAv", bufs=2)
        for ml in range(4):
            m = g * 4 + ml
            col = b * T + m * 128
            for k in range(KT):
                nc.tensor.matmul(
                    ps[:, ml * 128 : (ml + 1) * 128],
                    x_sb[k][:, col : col + 128],
                    cs["wv"][:, k * COLS : (k + 1) * COLS],
                    # start=True clears has_written for the WHOLE bank, so only
                    # the very first matmul of the packed group may set it
                    start=(ml == 0 and k == 0),
                    stop=False,
                )
        nc.tensor.matmul(
            ps[:], cs["ones"][0:1, :], cs["bv4"][0:1, :], start=False, stop=True
        )
        vi = (b * NT + g * 4) * HL
        nc.vector.tensor_copy(
            out=cs["v"][:, vi : vi + 4 * HL, 0:D],
            in_=ps[:].rearrange("p (m h d) -> p (m h) d", h=HL, d=D),
        )

    def attention(b, n, ptp, psS, psY, nrm, cs, send_instrs):
        """Causal attention for query chunk n of batch b, both heads."""
        qcol = b * T + n * 512
        qTd, kTd = cs["qTd"], cs["kTd"]
        last = 4 * n + 3
        py = [
            psY.tile([VW, 512], F32, tag=f"psY{h}", name=f"psY{h}", bufs=1)
            for h in range(HL)
        ]
        pts = []

        def emit_av(m):
            pt, lo = pts[m]
            for h in range(HL):
                vi = (b * NT + m) * HL + h
                nc.tensor.matmul(
                    py[h][:, lo:512],
                    cs["v"][:, vi : vi + 1, :],
                    pt[:, h * 512 + lo : (h + 1) * 512],
                    start=(m == 0),
                    stop=(m == last),
                )

        for m in range(last + 1):
            j = m - 4 * n  # >= 0 on the block-diagonal
            lo = max(j, 0) * 128
            ps2 = psS.tile([128, 1024], F32, tag="psS2", name="psS2", bufs=2)
            pt = ptp.tile([128, 1024], BF16, tag="pt", name="pt", bufs=5)
            kcol = b * T + m * 128
            for h in range(HL):
                nc.tensor.matmul(
                    ps2[:, h * 512 + lo : (h + 1) * 512],
                    kTd[h * D : (h + 1) * D, kcol : kcol + 128],
                    qTd[h * D : (h + 1) * D, qcol + lo : qcol + 512],
                    start=True,
                    stop=True,
                )
            if lo == 0:
                nc.scalar.activation(pt[:], ps2[:], Exp, scale=float(SCALE))
            else:
                for h in range(HL):
                    nc.scalar.activation(
                        pt[:, h * 512 + lo : (h + 1) * 512],
                        ps2[:, h * 512 + lo : (h + 1) * 512],
                        Exp,
                        scale=float(SCALE),
                    )
            if j >= 0:
                # zero the strict upper triangle of the diagonal 128-block
                for h in range(HL):
                    blk = pt[:, h * 512 + lo : h * 512 + lo + 128]
                    nc.vector.tensor_tensor(blk, blk, cs["tri"][:], mult)
            pts.append((pt, lo))
            if m > 0:
                emit_av(m - 1)
        emit_av(last)

        # ship unnormalized y^T + r rows; receiver divides
        yn = [
            nrm.tile([VW, 512], BF16, tag=f"yn{h}", name=f"yn{h}", bufs=2)
            for h in range(HL)
        ]
        for h in range(HL):
            nc.vector.tensor_copy(out=yn[h][:], in_=py[h][:])
        for p in range(2):
            dst = 2 * n + p
            for h in range(HL):
                si = nc.sync.dma_start(
                    out=send[b][dst * SH + h * VW : dst * SH + (h + 1) * VW, :],
                    in_=yn[h][:, p * 256 : (p + 1) * 256],
                )
                send_instrs.append(si)

    def a2a(b):
        nc.gpsimd.collective_compute(
            "AllToAll",
            mybir.AluOpType.bypass,
            replica_groups=[list(range(NCORES))],
            ins=[send[b][:]],
            outs=[recv[b][:]],
        )

    def stage_c(b, cp, psC, cs):
        """Wo row-shard contraction for this core's 256 tokens of batch b."""
        rv = recv[b].rearrange("(s r) c -> r s c", s=NCORES)
        y_sb = cp.tile([128, KT * 256], BF16, tag="ysb", name=f"ysb{b}", bufs=2)
        # y columns: chunk k partitions 0-63 = sender k h0 dims, 64-127 = h1
        nc.scalar.dma_start(
            out=y_sb[0:64, :].rearrange("d (s c) -> d s c", s=NCORES),
            in_=rv[0:64, :, :],
        )
        nc.scalar.dma_start(
            out=y_sb[64:128, :].rearrange("d (s c) -> d s c", s=NCORES),
            in_=rv[VW + 0 : VW + 64, :, :],
        )
        # SBUF-destination DMAs need the partition dim outermost in the AP
        rv_s = recv[b].rearrange("(s r) c -> s r c", s=NCORES)
        rsb = cp.tile([16, 256], BF16, tag="rsb", name=f"rsb{b}", bufs=2)
        nc.scalar.dma_start(
            out=rsb[0:8, :].rearrange("s (o c) -> s o c", o=1),
            in_=rv_s[:, 64:65, :],
        )
        nc.scalar.dma_start(
            out=rsb[8:16, :].rearrange("s (o c) -> s o c", o=1),
            in_=rv_s[:, VW + 64 : VW + 65, :],
        )
        rf = cp.tile([16, 256], F32, tag="rf", name=f"rf{b}", bufs=2)
        nc.vector.tensor_copy(out=rf[:], in_=rsb[:])
        invf = cp.tile([16, 256], F32, tag="invf", name=f"invf{b}", bufs=2)
        nc.vector.reciprocal_approx_fast(out=invf[:], in_=rf[:])
        inv = cp.tile([16, 256], BF16, tag="inv", name=f"inv{b}", bufs=2)
        nc.vector.tensor_copy(out=inv[:], in_=invf[:])

        yn_sb = cp.tile([128, KT * 256], BF16, tag="ynsb", name=f"ynsb{b}", bufs=2)
        for k in range(KT):
            sc = psC.tile([128, 256], F32, tag="psN", name="psN", bufs=2)
            nc.tensor.matmul(
                sc[:], cs["emat"][:, k * 128 : (k + 1) * 128], inv[:],
                start=True, stop=True,
            )
            nc.vector.tensor_tensor(
                yn_sb[:, k * 256 : (k + 1) * 256],
                y_sb[:, k * 256 : (k + 1) * 256],
                sc[:],
                mult,
            )
        for r in range(2):
            for o in range(2):
                pc = psC.tile([128, 512], F32, tag="psC", name="psC", bufs=2)
                for k in range(KT):
                    nc.tensor.matmul(
                        pc[:],
                        yn_sb[:, k * 256 + r * 128 : k * 256 + (r + 1) * 128],
                        cs["wo"][:, k * C + o * 512 : k * C + (o + 1) * 512],
                        start=(k == 0),
                        stop=False,
                    )
                nc.tensor.matmul(
                    pc[:], cs["ones"][0:1, :], cs["bo"][0:1, o * 512 : (o + 1) * 512],
                    start=False, stop=True,
                )
                osb = cp.tile([128, 512], F32, tag="osb", name="osb", bufs=2)
                nc.vector.tensor_copy(out=osb[:], in_=pc[:])
                nc.sync.dma_start(
                    out=out[b * 256 + r * 128 : b * 256 + (r + 1) * 128,
                            o * 512 : (o + 1) * 512],
                    in_=osb[:],
                )

    with TileContext(nc) as tc:
        with tc.tile_pool(name="persist", bufs=1) as pp:
            cs = {}
            # warmup collective first on the gpsimd queue: starts the cross-core
            # barrier + CC init while projections run
            nc.gpsimd.collective_compute(
                "AllToAll",
                mybir.AluOpType.bypass,
                replica_groups=[list(range(NCORES))],
                ins=[wsend[:]],
                outs=[wrecv[:]],
            )
            # tiny constants first: a late-landing constant can head-block the
            # PE queue
            cs["ones"] = pp.tile([1, 128], BF16, tag="ones", name="ones")
            nc.vector.memset(cs["ones"][:], 1.0)
            cs["bqk"] = pp.tile([COLS, 2], F32, tag="bqk", name="bqk")
            nc.sync.dma_start(out=cs["bqk"][:], in_=bqk[:])
            cs["bv4"] = pp.tile([1, 512], BF16, tag="bv4", name="bv4")
            nc.sync.dma_start(out=cs["bv4"][:], in_=bv4[:])
            cs["bo"] = pp.tile([1, C], BF16, tag="bo", name="bo")
            nc.scalar.dma_start(out=cs["bo"][:], in_=bo[:])
            cs["tri"] = pp.tile([128, 128], BF16, tag="tri", name="tri")
            nc.scalar.dma_start(out=cs["tri"][:], in_=tri[:])
            cs["emat"] = pp.tile([16, KT * 128], BF16, tag="emat", name="emat")
            nc.gpsimd.dma_start(out=cs["emat"][:], in_=emat[:])

            # weights: one per queue so each lands fast, before x
            cs["wq"] = pp.tile([128, KT * COLS], BF16, tag="wq", name="wq")
            cs["wk"] = pp.tile([128, KT * COLS], BF16, tag="wk", name="wk")
            cs["wv"] = pp.tile([128, KT * COLS], BF16, tag="wv", name="wv")
            nc.sync.dma_start(out=cs["wq"][:], in_=wq[:])
            nc.scalar.dma_start(out=cs["wk"][:], in_=wk[:])
            nc.gpsimd.dma_start(out=cs["wv"][:], in_=wv[:])

            with tc.tile_pool(name="xp", bufs=1) as xp:
                # x split per (token-chunk, k-tile) across 3 DMA trigger queues,
                # first-consumed chunks first
                x_sb = [
                    xp.tile([128, B * T], BF16, tag=f"x{k}", name=f"x{k}")
                    for k in range(KT)
                ]
                xq = [nc.sync, nc.scalar, nc.gpsimd]
                for c in range(4):
                    for k in range(KT):
                        xq[(c * KT + k) % 3].dma_start(
                            out=x_sb[k][:, c * 1024 : (c + 1) * 1024],
                            in_=xT[k * 128 : (k + 1) * 128, c * 1024 : (c + 1) * 1024],
                        )

                cs["qTd"] = pp.tile([128, B * T], BF16, tag="qTd", name="qTd")
                cs["kTd"] = pp.tile([128, B * T], BF16, tag="kTd", name="kTd")
                cs["v"] = pp.tile([128, B * NT * HL, VW], BF16, tag="v", name="v")
                nc.gpsimd.memset(cs["v"][:], 1.0)  # presets the ones columns

                # wo loaded last (not needed until stage C)
                cs["wo"] = pp.tile([128, KT * C], BF16, tag="wo", name="wo")
                nc.scalar.dma_start(out=cs["wo"][:], in_=wo[:])

                send_instrs = []
                with tc.tile_pool(name="psA", bufs=2, space="PSUM") as psA, \
                     tc.tile_pool(name="pt", bufs=5) as ptp, \
                     tc.tile_pool(name="psS", bufs=1, space="PSUM") as psS, \
                     tc.tile_pool(name="psY", bufs=1, space="PSUM") as psY, \
                     tc.tile_pool(name="nrm", bufs=2) as nrm:
                    for b in range(B):
                        # chunk-major: everything needing x-chunk c lands
                        # together; attention interleaves to cover DMA waits
                        proj_qk(b, 0, x_sb, psA, cs)
                        proj_qk(b, 1, x_sb, psA, cs)
                        proj_v(b, 0, x_sb, psA, cs)
                        proj_v(b, 1, x_sb, psA, cs)
                        attention(b, 0, ptp, psS, psY, nrm, cs, send_instrs)
                        attention(b, 1, ptp, psS, psY, nrm, cs, send_instrs)
                        proj_qk(b, 2, x_sb, psA, cs)
                        proj_qk(b, 3, x_sb, psA, cs)
                        proj_v(b, 2, x_sb, psA, cs)
                        proj_v(b, 3, x_sb, psA, cs)
                        attention(b, 2, ptp, psS, psY, nrm, cs, send_instrs)
                        attention(b, 3, ptp, psS, psY, nrm, cs, send_instrs)
                        a2a(b)

            with tc.tile_pool(name="cp", bufs=1) as cp, \
                 tc.tile_pool(name="psC", bufs=2, space="PSUM") as psC:
                stage_c(0, cp, psC, cs)
                stage_c(1, cp, psC, cs)
    nc.compile()
    return nc


def make_in_maps(x, mask, Wq, bq, Wk, bk, Wv, bv, Wo, bo):
    xT = np.ascontiguousarray(
        x.astype(np.float32).transpose(2, 0, 1).reshape(C, B * T)
    ).astype(NPBF16)
    tri_np = (
        np.arange(128)[None, :] >= np.arange(128)[:, None]
    ).astype(NPBF16)

    def pretile(w):
        # [C, width] -> [128, KT*width] with k-tile blocks along free axis
        width = w.shape[1]
        return np.ascontiguousarray(
            w.reshape(KT, 128, width).transpose(1, 0, 2).reshape(128, KT * width)
        )

    wo_b = pretile(Wo.astype(NPBF16))
    # emat[h*8+s, k*128+p] = 1 iff chunk k's partition p takes sender s, head h
    rows = np.arange(16)
    cols = np.arange(KT * 128)
    emat_np = (
        ((rows % 8)[:, None] == cols[None, :] // 128)
        & ((rows // 8)[:, None] == (cols[None, :] % 128) // 64)
    ).astype(NPBF16)
    bo_b = bo.reshape(1, C).astype(NPBF16)
    in_maps = []
    for c in range(NCORES):
        cslice = slice(c * COLS, (c + 1) * COLS)
        in_maps.append(
            {
                "xT": xT,
                "wq": pretile(Wq[:, cslice].astype(NPBF16)),
                "wk": pretile(Wk[:, cslice].astype(NPBF16)),
                "wv": pretile(Wv[:, cslice].astype(NPBF16)),
                "wo": wo_b,
                "bqk": np.stack([bq[cslice], bk[cslice]], axis=1).astype(
                    np.float32
                ),
                "bv4": np.tile(bv[cslice].astype(NPBF16), 4).reshape(1, 512),
                "bo": bo_b,
                "tri": tri_np,
                "emat": emat_np,
            }
        )
    return in_maps


_CACHED_NC = None


def run(inputs, trace=False, **kw):
    global _CACHED_NC
    if _CACHED_NC is None:
        _CACHED_NC = build_nc()
    in_maps = make_in_maps(**inputs)
    res = bass_utils.run_bass_kernel_spmd(
        _CACHED_NC, in_maps, core_ids=list(range(NCORES)), trace=trace, **kw
    )
    outs = [np.asarray(res.results[c]["out"]) for c in range(NCORES)]
    full = np.empty((B, T, C), np.float32)
    for j in range(NCORES):
        full[0, 256 * j : 256 * (j + 1)] = outs[j][0:256]
        full[1, 256 * j : 256 * (j + 1)] = outs[j][256:512]
    return full, res


def kernel(**inputs):
    full, _ = run(inputs, trace=False)
    return full
